# revision 1
# baseline (speedup 1.0000x reference)
"""Bass/Tile kernel for 3-scale local correlation (sparse_attention).

Per core: one batch sample, all 3 pyramid levels.

Algorithm per level:
  - aff[p, f] for each 2D pixel-tile (th x tw pixels on PSUM partitions) and
    dilated window position f=(fy, fx) computed by TensorE as a band Gram:
      stationary = tgt tile [c, th*tw], moving = padded-ref dilated tile [c, dh*dw]
  - window masking folded into the same PSUM accumulation as a rank-(th+tw)
    matmul of constant one-hot/band factors (adds -1000 outside each pixel's
    ws x ws window; exp -> 0)
  - ScalarE: e = exp(aff - 40) PSUM->SBUF (batched over PSUM banks)
  - VectorE: segmented reduces: row-max (conf), row-sum, col-sum
  - flow = (sum e * coord)/Z - (pixel + pad);  conf = m * exp(m - 40) / Z
"""

from contextlib import ExitStack

import numpy as np

import concourse.bass as bass
import concourse.bacc as bacc
import concourse.mybir as mybir
from concourse import tile

F32 = mybir.dt.float32
F16 = mybir.dt.float16
BF16 = mybir.dt.bfloat16

SHIFT = 40.0
MASKVAL = -1000.0
SLICE = 512  # PSUM bank pitch (fp32 elems)
TB = 4       # tiles per PSUM buffer

LEVELS = [
    dict(C=256, H=96, W=160, ws=9, th=8, tw=16),
    dict(C=512, H=48, W=80, ws=5, th=8, tw=16),
    dict(C=1024, H=24, W=40, ws=3, th=12, tw=10),
]


def level_params(lv):
    C, H, W, ws, th, tw = lv["C"], lv["H"], lv["W"], lv["ws"], lv["th"], lv["tw"]
    pad = ws // 2
    dh, dw = th + ws - 1, tw + ws - 1
    return dict(
        C=C, H=H, W=W, ws=ws, th=th, tw=tw, pad=pad,
        dh=dh, dw=dw, dil=dh * dw,
        nbh=H // th, nbw=W // tw, T=(H // th) * (W // tw),
        nch=C // 128, Hp=H + ws - 1, Wp=W + ws - 1,
        rank=th + tw, P=th * tw,
    )


PARAMS = [level_params(lv) for lv in LEVELS]


def make_consts():
    """Per-level constant arrays shared by every core."""
    consts = {}
    for i, p in enumerate(PARAMS):
        th, tw, ws, dh, dw, T = p["th"], p["tw"], p["ws"], p["dh"], p["dw"], p["T"]
        rank, P, pad = p["rank"], p["P"], p["pad"]
        um = np.zeros((rank, P), np.float16)
        vm = np.zeros((rank, dh * dw), np.float16)
        ty = np.arange(P) // tw
        tx = np.arange(P) % tw
        for j in range(th):
            um[j, :] = (ty == j).astype(np.float16)
            bad = ~((np.arange(dh) >= j) & (np.arange(dh) < j + ws))
            vm[j] = np.repeat(bad, dw) * np.float16(MASKVAL)
        for i2 in range(tw):
            um[th + i2, :] = (tx == i2).astype(np.float16)
            bad = ~((np.arange(dw) >= i2) & (np.arange(dw) < i2 + ws))
            vm[th + i2] = np.tile(bad, dh) * np.float16(MASKVAL)
        fybig = np.broadcast_to(
            np.tile(np.arange(dh, dtype=np.float32), T), (128, T * dh)
        ).copy()
        fxbig = np.broadcast_to(
            np.tile(np.arange(dw, dtype=np.float32), T), (128, T * dw)
        ).copy()
        tyc = np.zeros((128, 1), np.float32)
        txc = np.zeros((128, 1), np.float32)
        tyc[:P, 0] = ty + pad
        txc[:P, 0] = tx + pad
        consts[f"um{i}"] = um
        consts[f"vm{i}"] = vm
        consts[f"fybig{i}"] = fybig
        consts[f"fxbig{i}"] = fxbig
        consts[f"tyc{i}"] = tyc
        consts[f"txc{i}"] = txc
    return consts


def prep_core_inputs(tgts, refs):
    """Per-sample tensors: tgt as-is, refs zero-padded. All fp32."""
    d = {}
    for i, p in enumerate(PARAMS):
        pad = p["pad"]
        d[f"tgt{i}"] = np.ascontiguousarray(tgts[i])
        d[f"ref{i}p"] = np.pad(refs[i], ((0, 0), (pad, pad), (pad, pad)))
    d.update(make_consts())
    return d


def build_nc():
    nc = bacc.Bacc("TRN2", target_bir_lowering=False, debug=False,
                   enable_asserts=False)

    dram_in = {}
    dram_out = {}
    for i, p in enumerate(PARAMS):
        dram_in[f"tgt{i}"] = nc.dram_tensor(
            f"tgt{i}", (p["C"], p["H"], p["W"]), F32, kind="ExternalInput")
        dram_in[f"ref{i}p"] = nc.dram_tensor(
            f"ref{i}p", (p["C"], p["Hp"], p["Wp"]), F32, kind="ExternalInput")
        dram_in[f"um{i}"] = nc.dram_tensor(
            f"um{i}", (p["rank"], p["P"]), F16, kind="ExternalInput")
        dram_in[f"vm{i}"] = nc.dram_tensor(
            f"vm{i}", (p["rank"], p["dil"]), F16, kind="ExternalInput")
        dram_in[f"fybig{i}"] = nc.dram_tensor(
            f"fybig{i}", (128, p["T"] * p["dh"]), F32, kind="ExternalInput")
        dram_in[f"fxbig{i}"] = nc.dram_tensor(
            f"fxbig{i}", (128, p["T"] * p["dw"]), F32, kind="ExternalInput")
        dram_in[f"tyc{i}"] = nc.dram_tensor(
            f"tyc{i}", (128, 1), F32, kind="ExternalInput")
        dram_in[f"txc{i}"] = nc.dram_tensor(
            f"txc{i}", (128, 1), F32, kind="ExternalInput")
        for nm in ("fy", "fx", "cf"):
            dram_out[f"{nm}{i}"] = nc.dram_tensor(
                f"{nm}{i}", (p["H"], p["W"]), F32, kind="ExternalOutput")

    with ExitStack() as ctx:
        tc = ctx.enter_context(tile.TileContext(nc, trace_sim=False))
        img_pool = ctx.enter_context(tc.tile_pool(name="img", bufs=1))
        const_pool = ctx.enter_context(tc.tile_pool(name="const", bufs=1))
        psum_pool = ctx.enter_context(tc.tile_pool(name="psum", bufs=2, space="PSUM"))
        e_pool = ctx.enter_context(tc.tile_pool(name="e", bufs=3))
        stat_pool = ctx.enter_context(tc.tile_pool(name="stat", bufs=1))
        fin_pool = ctx.enter_context(tc.tile_pool(name="fin", bufs=1))

        for i, p in enumerate(PARAMS):
            C, H, W, ws = p["C"], p["H"], p["W"], p["ws"]
            th, tw, pad, dh, dw, dil = (p["th"], p["tw"], p["pad"], p["dh"],
                                        p["dw"], p["dil"])
            nbh, nbw, T, nch = p["nbh"], p["nbw"], p["T"], p["nch"]
            Hp, Wp, rank, P = p["Hp"], p["Wp"], p["rank"], p["P"]

            # ---- loads (fp32 -> fp16 cast during DMA) ----
            tgt_t = img_pool.tile([128, nch, H, W], F16, tag="tgt")
            ref_t = img_pool.tile([128, nch, Hp, Wp], F16, tag="ref")
            nc.gpsimd.dma_start(
                out=tgt_t[:, :, :, :],
                in_=dram_in[f"tgt{i}"].ap().rearrange("(k p) h w -> p k h w", p=128))
            nc.gpsimd.dma_start(
                out=ref_t[:, :, :, :],
                in_=dram_in[f"ref{i}p"].ap().rearrange("(k p) h w -> p k h w", p=128))

            um_t = const_pool.tile([rank, P], F16, tag=f"um{i}")
            vm_t = const_pool.tile([rank, dil], F16, tag=f"vm{i}")
            fyb_t = const_pool.tile([128, T * dh], F32, tag=f"fyb{i}")
            fxb_t = const_pool.tile([128, T * dw], F32, tag=f"fxb{i}")
            tyc_t = const_pool.tile([128, 1], F32, tag=f"tyc{i}")
            txc_t = const_pool.tile([128, 1], F32, tag=f"txc{i}")
            for nm, t in (("um", um_t), ("vm", vm_t), ("fybig", fyb_t),
                          ("fxbig", fxb_t), ("tyc", tyc_t), ("txc", txc_t)):
                nc.sync.dma_start(out=t[...], in_=dram_in[f"{nm}{i}"].ap())

            mrows = stat_pool.tile([128, T * dh], F32, tag=f"mrows{i}")
            rsum = stat_pool.tile([128, T * dh], F32, tag=f"rsum{i}")
            csum = stat_pool.tile([128, T * dw], F32, tag=f"csum{i}")

            # ---- per PSUM batch: matmuls + max + exp + row/col sums ----
            nbatches = (T + TB - 1) // TB
            for b in range(nbatches):
                t0 = b * TB
                nb = min(TB, T - t0)
                ptile = psum_pool.tile([128, TB * SLICE], F32, tag="psum")
                for s in range(nb):
                    t = t0 + s
                    bh, bw = divmod(t, nbw)
                    out_sl = ptile[:P, s * SLICE: s * SLICE + dil]
                    nc.tensor.matmul(out_sl, um_t[:, :], vm_t[:, :],
                                     start=True, stop=False)
                    for k in range(nch):
                        lhsT = tgt_t[:, k, bh * th:(bh + 1) * th,
                                     bw * tw:(bw + 1) * tw]
                        rhs = ref_t[:, k, bh * th: bh * th + dh,
                                    bw * tw: bw * tw + dw]
                        nc.tensor.matmul(out_sl, lhsT, rhs,
                                         start=False, stop=(k == nch - 1))

                pview = (ptile[:P, :]
                         .rearrange("p (t s) -> p t s", s=SLICE)[:, :nb, :dil]
                         .rearrange("p t (fy fx) -> p t fy fx", fy=dh))
                # row-max of masked aff (for conf)
                nc.vector.tensor_reduce(
                    out=(mrows[:P, t0 * dh:(t0 + nb) * dh]
                         .rearrange("p (t fy) -> p t fy", fy=dh)),
                    in_=pview, axis=mybir.AxisListType.X, op=mybir.AluOpType.max)
                # exp
                e_t = e_pool.tile([128, TB * dil], BF16, tag="e")
                eview = (e_t[:P, : nb * dil]
                         .rearrange("p (t f) -> p t f", f=dil)
                         .rearrange("p t (fy fx) -> p t fy fx", fy=dh))
                nc.scalar.activation(
                    eview, pview, mybir.ActivationFunctionType.Exp,
                    bias=-SHIFT, scale=1.0)
                # row sums
                nc.vector.tensor_reduce(
                    out=(rsum[:P, t0 * dh:(t0 + nb) * dh]
                         .rearrange("p (t fy) -> p t fy", fy=dh)),
                    in_=eview, axis=mybir.AxisListType.X, op=mybir.AluOpType.add)
                # col sums (transposed view of e)
                nc.vector.tensor_reduce(
                    out=(csum[:P, t0 * dw:(t0 + nb) * dw]
                         .rearrange("p (t fx) -> p t fx", fx=dw)),
                    in_=eview.rearrange("p t fy fx -> p t fx fy"),
                    axis=mybir.AxisListType.X, op=mybir.AluOpType.add)

            # ---- finals ----
            m_f = fin_pool.tile([128, T], F32, tag=f"m{i}")
            z_f = fin_pool.tile([128, T], F32, tag=f"z{i}")
            rz_f = fin_pool.tile([128, T], F32, tag=f"rz{i}")
            em_f = fin_pool.tile([128, T], F32, tag=f"em{i}")
            sy_f = fin_pool.tile([128, T], F32, tag=f"sy{i}")
            sx_f = fin_pool.tile([128, T], F32, tag=f"sx{i}")
            fy_o = fin_pool.tile([128, T], F32, tag=f"fyo{i}")
            fx_o = fin_pool.tile([128, T], F32, tag=f"fxo{i}")
            cf_o = fin_pool.tile([128, T], F32, tag=f"cfo{i}")
            scr = stat_pool.tile([128, T * max(dh, dw)], F32, tag=f"scr{i}")

            mr_v = mrows[:P, :].rearrange("p (t fy) -> p t fy", fy=dh)
            rs_v = rsum[:P, :].rearrange("p (t fy) -> p t fy", fy=dh)
            cs_v = csum[:P, :].rearrange("p (t fx) -> p t fx", fx=dw)

            nc.vector.tensor_reduce(out=m_f[:P, :], in_=mr_v,
                                    axis=mybir.AxisListType.X,
                                    op=mybir.AluOpType.max)
            nc.vector.tensor_reduce(out=z_f[:P, :], in_=rs_v,
                                    axis=mybir.AxisListType.X,
                                    op=mybir.AluOpType.add)
            # Sy = sum(rsum * fy) ; Sx = sum(csum * fx)
            nc.vector.tensor_mul(scr[:P, :T * dh], rsum[:P, :], fyb_t[:P, :])
            nc.vector.tensor_reduce(
                out=sy_f[:P, :],
                in_=scr[:P, :T * dh].rearrange("p (t fy) -> p t fy", fy=dh),
                axis=mybir.AxisListType.X, op=mybir.AluOpType.add)
            nc.vector.tensor_mul(scr[:P, :T * dw], csum[:P, :], fxb_t[:P, :])
            nc.vector.tensor_reduce(
                out=sx_f[:P, :],
                in_=scr[:P, :T * dw].rearrange("p (t fx) -> p t fx", fx=dw),
                axis=mybir.AxisListType.X, op=mybir.AluOpType.add)

            nc.vector.reciprocal(rz_f[:P, :], z_f[:P, :])
            nc.scalar.activation(em_f[:P, :], m_f[:P, :],
                                 mybir.ActivationFunctionType.Exp,
                                 bias=-SHIFT, scale=1.0)
            # conf = m * em * rz
            nc.vector.tensor_mul(cf_o[:P, :], m_f[:P, :], em_f[:P, :])
            nc.vector.tensor_mul(cf_o[:P, :], cf_o[:P, :], rz_f[:P, :])
            # flow = S*rz - (t + pad)
            nc.vector.tensor_mul(fy_o[:P, :], sy_f[:P, :], rz_f[:P, :])
            nc.vector.tensor_scalar_sub(fy_o[:P, :], fy_o[:P, :], tyc_t[:P, :])
            nc.vector.tensor_mul(fx_o[:P, :], sx_f[:P, :], rz_f[:P, :])
            nc.vector.tensor_scalar_sub(fx_o[:P, :], fx_o[:P, :], txc_t[:P, :])

            # ---- outputs ----
            for nm, t in (("fy", fy_o), ("fx", fx_o), ("cf", cf_o)):
                nc.sync.dma_start(
                    out=dram_out[f"{nm}{i}"].ap().rearrange(
                        "(bh ty) (bw tx) -> (ty tx) bh bw", ty=th, tx=tw),
                    in_=t[:P, :].rearrange("p (bh bw) -> p bh bw", bw=nbw))

    nc.compile()
    return nc


def assemble_outputs(results):
    """results: list of 8 per-core dicts -> the 6 full outputs."""
    flows, confs = [], []
    for i, p in enumerate(PARAMS):
        fy = np.stack([r[f"fy{i}"] for r in results])
        fx = np.stack([r[f"fx{i}"] for r in results])
        cf = np.stack([r[f"cf{i}"] for r in results])
        flows.append(np.stack([fy, fx], axis=-1))
        confs.append(cf[..., None])
    return (*flows, *confs)


# ---------------------------------------------------------------------------
# harness entry point
# ---------------------------------------------------------------------------
from concourse.bass_utils import run_bass_kernel_spmd as _run_spmd

_NC_CACHE = None


def _get_nc():
    global _NC_CACHE
    if _NC_CACHE is None:
        _NC_CACHE = build_nc()
    return _NC_CACHE


def kernel(tgt0, tgt1, tgt2, ref0, ref1, ref2):
    """Full (unsharded) inputs -> full outputs.

    Shards the batch dim (8) across the 8 NeuronCores; each core computes
    all three pyramid levels for its sample.
    """
    nc = _get_nc()
    tgts_all = [np.asarray(tgt0), np.asarray(tgt1), np.asarray(tgt2)]
    refs_all = [np.asarray(ref0), np.asarray(ref1), np.asarray(ref2)]
    B = tgts_all[0].shape[0]
    assert B == 8
    in_maps = []
    for b in range(B):
        in_maps.append(prep_core_inputs([t[b] for t in tgts_all],
                                        [r[b] for r in refs_all]))
    res = _run_spmd(nc, in_maps, core_ids=list(range(8)))
    return assemble_outputs(res.results)
